# revision 8
# baseline (speedup 1.0000x reference)
"""GQA causal attention (B=2, S=2048, H=2048, 16 q-heads / 4 kv-groups) on 8 trn2 cores.

Sharding: core = batch * 4 + group (data-parallel on batch, tensor-parallel on
whole KV groups).  Each core computes q/k/v projections for its group, causal
attention for its 4 query heads, and a row-parallel partial of the output
projection.  The host transposes and sums the 4 group partials per batch.

Device-side layout is feature-on-partition throughout (q^T, k^T, scores^T)
so every matmul consumes natural layouts:
  qT[d, s]   = Wq_g^T X^T        (lhsT=Wq_g slice, rhs=X^T slice)
  kT[d, s]   = Wk_g^T X^T
  vT[d, s]   = Wv_g^T X^T  -> PE-transposed into vN[s, d] 128x128 chunks
  ST[sj, si] = kT^T qT           (scores, transposed)
  PT         = exp(scale*ST) * causal01
  OT[d, si]  = vN^T PT           (accumulated over sj chunks)
  rowsum[si] = ones^T (sum_J PT) ; OT *= 1/rowsum (broadcast via SBUF DMA)
  YT[n, si]  = Wo_g^T OT         (partial of the o_proj, row-parallel)
Causal structure: for query tile si=[512I,512I+512) only sj chunks J<=4I+3
are computed; the 4 diagonal chunks are masked with a precomputed 0/1 tile.
Softmax skips max-subtraction (|scale*scores| < ~6 here) and normalizes at OT.
"""

import math
from contextlib import ExitStack

import numpy as np

import concourse.bass as bass
import concourse.mybir as mybir
import concourse.tile as tile
from concourse import bacc
from concourse.bass_utils import run_bass_kernel_spmd
from concourse.masks import make_identity

B, S, H = 2, 2048, 2048
NH, G = 16, 4
HD = 128
REP = NH // G          # 4 query heads per group
D4 = REP * HD          # 512 q dims per group
P = 128
HC = H // P            # 16 contraction chunks over H
TN = 512               # si tile width
NI = S // TN           # 4 si tiles
SCALE = 1.0 / math.sqrt(HD)

F32 = mybir.dt.float32
F32R = mybir.dt.float32r
EXP = mybir.ActivationFunctionType.Exp

LAST_RESULT = None
_MODULE_CACHE = {}


def _r(ap):
    return ap.bitcast(F32R)


def _f(ap):
    return ap.bitcast(F32)


def _build_module():
    nc = bacc.Bacc("TRN2", target_bir_lowering=False, debug=False, num_devices=8)
    xt = nc.dram_tensor("xt", [H, S], F32, kind="ExternalInput")
    wq = nc.dram_tensor("wq", [H, D4], F32, kind="ExternalInput")
    wk = nc.dram_tensor("wk", [H, HD], F32, kind="ExternalInput")
    wv = nc.dram_tensor("wv", [H, HD], F32, kind="ExternalInput")
    wo = nc.dram_tensor("wo", [D4, H], F32, kind="ExternalInput")
    bq = nc.dram_tensor("bq", [D4], F32, kind="ExternalInput")
    bk = nc.dram_tensor("bk", [HD], F32, kind="ExternalInput")
    bv = nc.dram_tensor("bv", [HD], F32, kind="ExternalInput")
    # mask01[p, 384 - 128*j0 + f] = 1.0 where (128*j0 + p) <= f else 0.0
    mask = nc.dram_tensor("mask", [P, 896], F32, kind="ExternalInput")
    ones = nc.dram_tensor("ones", [P], F32, kind="ExternalInput")
    yt = nc.dram_tensor("yt", [H, S], F32, kind="ExternalOutput")

    with ExitStack() as ctx:
        ctx.enter_context(nc.allow_low_precision(
            reason="float32r tiles feed the PE; fp32r rounding is intended"))
        tc = ctx.enter_context(tile.TileContext(nc))
        const = ctx.enter_context(tc.tile_pool(name="const", bufs=1))
        state = ctx.enter_context(tc.tile_pool(name="state", bufs=1))
        xpool = ctx.enter_context(tc.tile_pool(name="xpool", bufs=16))
        qpool = ctx.enter_context(tc.tile_pool(name="qpool", bufs=2))
        vtpool = ctx.enter_context(tc.tile_pool(name="vtpool", bufs=2))
        ptpool = ctx.enter_context(tc.tile_pool(name="ptpool", bufs=4))
        rapool = ctx.enter_context(tc.tile_pool(name="rapool", bufs=2))
        rcpool = ctx.enter_context(tc.tile_pool(name="rcpool", bufs=2))
        bcpool = ctx.enter_context(tc.tile_pool(name="bcpool", bufs=2))
        otpool = ctx.enter_context(tc.tile_pool(name="otpool", bufs=2))
        ypool = ctx.enter_context(tc.tile_pool(name="ypool", bufs=2))
        # PSUM budget is 8 banks: shared [128,512] "mm" tag 5 + tr 1 + row 1 + bc 1
        ps_mm = ctx.enter_context(tc.tile_pool(name="ps_mm", bufs=5, space="PSUM"))
        ps_tr = ctx.enter_context(tc.tile_pool(name="ps_tr", bufs=1, space="PSUM"))
        ps_row = ctx.enter_context(tc.tile_pool(name="ps_row", bufs=1, space="PSUM"))
        ps_bc = ctx.enter_context(tc.tile_pool(name="ps_bc", bufs=1, space="PSUM"))

        wq_sb = const.tile([P, HC, D4], F32R, tag="wq")
        nc.sync.dma_start(out=wq_sb, in_=wq[:].rearrange("(c p) d -> p c d", p=P).bitcast(F32R))
        wk_sb = const.tile([P, HC, HD], F32R, tag="wk")
        nc.sync.dma_start(out=wk_sb, in_=wk[:].rearrange("(c p) d -> p c d", p=P).bitcast(F32R))
        wv_sb = const.tile([P, HC, HD], F32R, tag="wv")
        nc.sync.dma_start(out=wv_sb, in_=wv[:].rearrange("(c p) d -> p c d", p=P).bitcast(F32R))
        wo_sb = const.tile([P, REP, H], F32R, tag="wo")
        nc.sync.dma_start(out=wo_sb, in_=wo[:].rearrange("(c p) n -> p c n", p=P).bitcast(F32R))
        mask_sb = const.tile([P, 896], F32, tag="mask")
        nc.sync.dma_start(out=mask_sb, in_=mask[:])
        ident = const.tile([P, P], F32, tag="ident")
        make_identity(nc, ident)
        ones_col = const.tile([P, 1], F32R, tag="ones_col")
        ones_row = const.tile([1, P], F32R, tag="ones_row")
        with nc.allow_non_contiguous_dma(reason="tiny ones loads"):
            nc.sync.dma_start(out=ones_col, in_=ones[:].rearrange("(c p) -> p c", p=P).bitcast(F32R))
            nc.sync.dma_start(out=ones_row, in_=ones[None, :].bitcast(F32R))
        bq_sb = const.tile([P, REP], F32, tag="bq")
        bk_sb = const.tile([P, 1], F32, tag="bk")
        bv_sb = const.tile([P, 1], F32, tag="bv")
        with nc.allow_non_contiguous_dma(reason="tiny per-partition bias loads"):
            nc.sync.dma_start(out=bq_sb, in_=bq[:].rearrange("(c p) -> p c", p=P))
            nc.sync.dma_start(out=bk_sb, in_=bk[:].rearrange("(c p) -> p c", p=P))
            nc.sync.dma_start(out=bv_sb, in_=bv[:].rearrange("(c p) -> p c", p=P))

        kt_sb = state.tile([P, S], F32R, tag="kt")          # [d, sj]
        vn_sb = state.tile([P, HC, HD], F32R, tag="vn")     # [sj%128, J, d]

        for I in range(NI):
            s0 = I * TN
            # ---- projections for s in [s0, s0+TN) ----
            xts = []
            for h in range(HC):
                xt_t = xpool.tile([P, TN], F32R, tag="xt")
                nc.sync.dma_start(out=xt_t, in_=xt[h * P:(h + 1) * P, s0:s0 + TN].bitcast(F32R))
                xts.append(xt_t)
            qt_t = qpool.tile([P, REP, TN], F32R, tag="qt")  # [d, r, si]
            for r in range(REP):
                pq = ps_mm.tile([P, TN], F32, tag="mm")
                for h in range(HC):
                    nc.tensor.matmul(
                        pq, wq_sb[:, h, r * HD:(r + 1) * HD], xts[h],
                        start=(h == 0), stop=(h == HC - 1),
                    )
                nc.any.tensor_scalar_add(
                    out=qt_t[:, r, :], in0=pq, scalar1=bq_sb[:, r:r + 1])
            pk = ps_mm.tile([P, TN], F32, tag="mm")
            for h in range(HC):
                nc.tensor.matmul(pk, wk_sb[:, h, :], xts[h],
                                 start=(h == 0), stop=(h == HC - 1))
            nc.any.tensor_scalar_add(out=kt_sb[:, s0:s0 + TN], in0=pk, scalar1=bk_sb)
            pv = ps_mm.tile([P, TN], F32, tag="mm")
            for h in range(HC):
                nc.tensor.matmul(pv, wv_sb[:, h, :], xts[h],
                                 start=(h == 0), stop=(h == HC - 1))
            vt_t = vtpool.tile([P, TN], F32, tag="vt")
            nc.any.tensor_scalar_add(out=vt_t, in0=pv, scalar1=bv_sb)
            for jj in range(TN // P):
                J = I * (TN // P) + jj
                ptr = ps_tr.tile([P, P], F32, tag="tr")
                nc.tensor.transpose(ptr, vt_t[:, jj * P:(jj + 1) * P], ident)
                nc.any.tensor_copy(out=vn_sb[:, J, :], in_=ptr)

            # ---- attention for si tile I, heads r=0..3 ----
            nJ = 4 * I + 4
            ot_t = otpool.tile([P, REP, TN], F32R, tag="ot")  # [d, r, si]
            for r in range(REP):
                ot_ps = ps_mm.tile([P, TN], F32, tag="mm")
                ra = rapool.tile([P, TN], F32R, tag="ra")
                for J in range(nJ):
                    st_ps = ps_mm.tile([P, TN], F32, tag="mm")
                    nc.tensor.matmul(
                        st_ps, kt_sb[:, J * P:(J + 1) * P], qt_t[:, r, :],
                        start=True, stop=True,
                    )
                    pt = ptpool.tile([P, TN], F32R, tag="pt")
                    nc.scalar.activation(out=pt, in_=st_ps, func=EXP, scale=SCALE)
                    if J >= 4 * I:
                        j0 = J - 4 * I
                        o = 384 - 128 * j0
                        nc.vector.tensor_mul(
                            out=pt, in0=pt, in1=mask_sb[:, o:o + TN])
                    nc.tensor.matmul(ot_ps, vn_sb[:, J, :], pt,
                                     start=(J == 0), stop=(J == nJ - 1))
                    if J == 0:
                        nc.vector.tensor_copy(out=ra, in_=pt)
                    else:
                        nc.vector.tensor_add(out=ra, in0=ra, in1=pt)
                row_ps = ps_row.tile([1, TN], F32, tag="row")
                nc.tensor.matmul(row_ps, ones_col, ra, start=True, stop=True)
                rc = rcpool.tile([1, TN], F32R, tag="rc")
                nc.vector.reciprocal(out=rc, in_=row_ps)
                bc = ps_bc.tile([P, TN], F32, tag="bc")
                nc.tensor.matmul(bc, ones_row, rc, start=True, stop=True)
                bc_sb = bcpool.tile([P, TN], F32, tag="bc_sb")
                nc.any.tensor_copy(out=bc_sb, in_=bc)
                nc.vector.tensor_mul(out=ot_t[:, r, :], in0=ot_ps, in1=bc_sb)

            # ---- o_proj partial for si tile I ----
            for n in range(HC):
                yp = ps_mm.tile([P, TN], F32, tag="mm")
                for c in range(REP):
                    nc.tensor.matmul(
                        yp, wo_sb[:, c, n * P:(n + 1) * P], ot_t[:, c, :],
                        start=(c == 0), stop=(c == REP - 1),
                    )
                yt_t = ypool.tile([P, TN], F32, tag="yt")
                nc.any.tensor_copy(out=yt_t, in_=yp)
                nc.sync.dma_start(out=yt[n * P:(n + 1) * P, s0:s0 + TN], in_=yt_t)

    nc.compile()
    return nc


def _get_module():
    if "nc" not in _MODULE_CACHE:
        _MODULE_CACHE["nc"] = _build_module()
    return _MODULE_CACHE["nc"]


def kernel(**inputs):
    global LAST_RESULT
    X = np.ascontiguousarray(np.asarray(inputs["X"], dtype=np.float32))
    Wq = np.asarray(inputs["Wq"], dtype=np.float32)
    Wk = np.asarray(inputs["Wk"], dtype=np.float32)
    Wv = np.asarray(inputs["Wv"], dtype=np.float32)
    Wo = np.asarray(inputs["Wo"], dtype=np.float32)
    bq = np.asarray(inputs["bq"], dtype=np.float32)
    bk = np.asarray(inputs["bk"], dtype=np.float32)
    bv = np.asarray(inputs["bv"], dtype=np.float32)
    bo = np.asarray(inputs["bo"], dtype=np.float32)
    cm = np.asarray(inputs["casual_mask"], dtype=np.float32)[0, 0]

    mask_big = np.zeros((P, 896), dtype=np.float32)
    mask_big[:, 384:] = 1.0 - cm[0:TN, 0:P].T  # [p, f] = (p <= f)

    nc = _get_module()
    in_maps = []
    for b in range(B):
        for g in range(G):
            in_maps.append({
                "xt": np.ascontiguousarray(X[b].T),
                "wq": np.ascontiguousarray(Wq[:, g * D4:(g + 1) * D4]),
                "wk": np.ascontiguousarray(Wk[:, g * HD:(g + 1) * HD]),
                "wv": np.ascontiguousarray(Wv[:, g * HD:(g + 1) * HD]),
                "wo": np.ascontiguousarray(Wo[g * D4:(g + 1) * D4, :]),
                "bq": np.ascontiguousarray(bq[g * D4:(g + 1) * D4]),
                "bk": np.ascontiguousarray(bk[g * HD:(g + 1) * HD]),
                "bv": np.ascontiguousarray(bv[g * HD:(g + 1) * HD]),
                "mask": mask_big,
                "ones": np.ones((P,), dtype=np.float32),
            })
    res = run_bass_kernel_spmd(nc, in_maps, core_ids=list(range(8)))
    LAST_RESULT = res
    Y = np.empty((B, S, H), dtype=np.float32)
    for b in range(B):
        acc = res.results[b * G + 0]["yt"].T.astype(np.float32).copy()
        for g in range(1, G):
            acc += res.results[b * G + g]["yt"].T
        Y[b] = acc + bo[None, :]
    return Y
